# revision 29
# baseline (speedup 1.0000x reference)
"""Multi-head attention (N=2, T=2048, D=1024, H=16, dk=dv=64) on 8 TRN2 cores.

Sharding: tensor-parallel over heads. Core p computes heads {2p, 2p+1}
(a 128-wide slice of the QKV projections and of WO's rows), producing a
partial output [2, 2048, 1024]; the host sums the 8 partials and adds bO
(row-parallel linear => sum-reduce unshard).

Device algorithm (per core, per batch n):
  1. qT = (WQp/8).T @ Q.T   [128, 2048]   (scale 1/sqrt(dk) folded into WQp)
     kT = WKp.T @ K.T       [128, 2048]
     v  = V @ WVp           [128part(l-tile), 16, 2*(dk+1)] with a ones
          column appended per head (gives the softmax denominator for free)
  2. scores in "KQ" orientation: S^T[l, q] = kT.T(l-tile) @ qT(q-chunk)
     (per head, K=64 contraction)
  3. E = exp(S^T) on ScalarE, PSUM -> SBUF bf16.  Unit-variance scores =>
     no max-subtraction needed (max |S| ~ 5, exp safe).
  4. attnT_aug[dv+1, q] += v_aug.T(l-tile) @ E  accumulated over l-tiles in
     PSUM; row dv holds sum(exp) = softmax denominator.
  5. normalize: att[dv, q] = attnT * bcast(1/den) (DVE mul; denominator
     broadcast across partitions via a K=1 PE matmul)
  6. O^T-partial: out[q-tile, :] = att[:, q-tile].T @ WOp, written fp16.

All wire/matmul dtypes are bf16 (PSUM accumulation fp32); ScalarE's exp
(16.8M elems, ~138us) and the PE stream (~143us) are co-critical, so the
schedule keeps both dense: every input DMA is issued >= ~10 l-tiles ahead
of the matmul that consumes it, and projection/out-proj/normalize work is
drip-fed into the attention l-loop's PE slack.
"""

import math
import numpy as np
from contextlib import ExitStack
from collections import deque

import concourse.bass as bass
import concourse.tile as tile
from concourse import bacc, mybir
from concourse.bass_utils import run_bass_kernel_spmd

N_CORES = 8
NB, T, D = 2, 2048, 1024
HEADS, DK = 16, 64
HP = 2 * DK          # per-core head-pair width = 128
QC = 512             # query-chunk (matmul moving free dim)
NQC = T // QC        # 4
LTS = 128            # key/l tile (PE partition dim)
NLT = T // LTS       # 16
CK = 128             # contraction chunk for projections
NCK = D // CK        # 8
VW = DK + 1          # v columns per head incl. ones column

F32 = mybir.dt.float32
BF16 = mybir.dt.bfloat16
FP16 = mybir.dt.float16
EXP = mybir.ActivationFunctionType.Exp


def build_program(mm_dt=BF16, out_dt=FP16):
    """Build + compile the SPMD program (identical on all 8 cores)."""
    nc = bacc.Bacc("TRN2", target_bir_lowering=False, debug=False,
                   num_devices=N_CORES)
    QT = nc.dram_tensor("QT", [NB, NCK, NQC, CK, QC], mm_dt,
                        kind="ExternalInput").ap()
    KT = nc.dram_tensor("KT", [NB, NCK, NQC, CK, QC], mm_dt,
                        kind="ExternalInput").ap()
    VT = nc.dram_tensor("VT", [NB, NCK, NQC, CK, QC], mm_dt,
                        kind="ExternalInput").ap()
    WQp = nc.dram_tensor("WQp", [D, HP], mm_dt, kind="ExternalInput").ap()
    WKp = nc.dram_tensor("WKp", [D, HP], mm_dt, kind="ExternalInput").ap()
    WVp = nc.dram_tensor("WVp", [D, HP], mm_dt, kind="ExternalInput").ap()
    WOp = nc.dram_tensor("WOp", [HP, D], mm_dt, kind="ExternalInput").ap()
    O = nc.dram_tensor("O", [NB, T, D], out_dt, kind="ExternalOutput").ap()

    with tile.TileContext(nc) as tc, ExitStack() as ctx:
        wpool = ctx.enter_context(tc.tile_pool(name="w", bufs=1))
        seq = ctx.enter_context(tc.tile_pool(name="seq", bufs=2))
        inp = ctx.enter_context(tc.tile_pool(name="inp", bufs=12))
        epool = ctx.enter_context(tc.tile_pool(name="e", bufs=6))
        apool = ctx.enter_context(tc.tile_pool(name="att", bufs=2))
        opool = ctx.enter_context(tc.tile_pool(name="o", bufs=3))
        ppool = ctx.enter_context(tc.tile_pool(name="pp", bufs=2, space="PSUM"))
        spool = ctx.enter_context(tc.tile_pool(name="ps", bufs=2, space="PSUM"))
        atpool = ctx.enter_context(tc.tile_pool(name="pa", bufs=1, space="PSUM"))

        # --- static SBUF: weights + constants ---
        wq_s = wpool.tile([CK, NCK, HP], mm_dt)
        wk_s = wpool.tile([CK, NCK, HP], mm_dt)
        wv_s = wpool.tile([CK, NCK, HP], mm_dt)
        wo_s = wpool.tile([HP, D], mm_dt)

        def load_w(w_s, W):
            nc.sync.dma_start(
                out=w_s, in_=W.rearrange("(k c) m -> c k m", c=CK))

        ones_col = wpool.tile([1, DK], F32, name="ones_col")
        nc.vector.memset(ones_col, 1.0)
        ones_col_r = wpool.tile([1, DK], mm_dt, name="ones_col_r")
        nc.vector.tensor_copy(ones_col_r, ones_col)
        ones_lts = wpool.tile([LTS, NLT, 1], F32, name="ones_lts")
        nc.vector.memset(ones_lts, 1.0)

        # --- staged input chunks (DMA decoupled from the matmuls) ---
        stage = {}

        def dma_thunks(src, n, qc, key, bufs=96):
            """8 thunks, each DMAs one [CK, QC] block of (src, n, qc)."""
            def t(ck):
                def f():
                    cin = inp.tile([CK, QC], mm_dt, tag="cin", bufs=bufs,
                                   name="cin")
                    nc.sync.dma_start(out=cin, in_=src[n, ck, qc])
                    stage[(key, ck)] = cin
                return f
            return [t(ck) for ck in range(NCK)]

        def qk_mm_thunks(w_s, dst, qc, key, eng=None):
            """Projection matmuls in 2 bursts of 4 + a PSUM->SBUF cast.
            `eng` picks the cast engine: ScalarE inside the dense projection
            blocks (no exp there, and it breaks the PE<->DVE ping-pong
            through the 2-buffer PSUM ring), DVE inside the l-loops."""
            box = {}
            def burst(cks):
                def f():
                    if 0 in cks:
                        box["ps"] = ppool.tile([HP, QC], F32, tag="pp",
                                               name="ps_proj")
                    for ck in cks:
                        nc.tensor.matmul(box["ps"], lhsT=w_s[:, ck, :],
                                         rhs=stage.pop((key, ck)),
                                         start=(ck == 0),
                                         stop=(ck == NCK - 1))
                return f
            def cast():
                dst_ap = dst[:, qc * QC:(qc + 1) * QC]
                if eng == "scalar":
                    nc.scalar.copy(dst_ap, box.pop("ps"))
                else:
                    nc.vector.tensor_copy(dst_ap, box.pop("ps"))
            return [burst(range(0, 4)), burst(range(4, 8)), cast]

        def v_mm_thunks(v_sb, c, key, eng=None):
            """v natural-layout projection for token-chunk c (4 l-tiles)."""
            cins = {}
            def grab():
                for ck in range(NCK):
                    cins[ck] = stage.pop((key, ck))
            def t(j):
                def f():
                    lt = c * (QC // LTS) + j
                    pv = ppool.tile([LTS, HP], F32, tag="pp", name="pv")
                    for ck in range(NCK):
                        nc.tensor.matmul(
                            pv, lhsT=cins[ck][:, j * LTS:(j + 1) * LTS],
                            rhs=wv_s[:, ck, :],
                            start=(ck == 0), stop=(ck == NCK - 1))
                    if eng == "scalar":
                        nc.scalar.copy(v_sb[:, lt, 0:DK], pv[:, 0:DK])
                        nc.scalar.copy(v_sb[:, lt, VW:VW + DK], pv[:, DK:HP])
                    else:
                        nc.vector.tensor_copy(v_sb[:, lt, 0:DK], pv[:, 0:DK])
                        nc.vector.tensor_copy(v_sb[:, lt, VW:VW + DK],
                                              pv[:, DK:HP])
                return f
            return [grab] + [t(j) for j in range(QC // LTS)]

        def scores_pair(qT_sb, kT_sb, qc, lt):
            """S^T for both heads of (q-chunk, l-tile) into one 2-bank PSUM
            tile; single wide exp; returns E tile [LTS, 2*QC]."""
            ss = spool.tile([LTS, 2 * QC], F32, tag="ss", name="ss")
            for h in range(2):
                nc.tensor.matmul(
                    ss[:, h * QC:(h + 1) * QC],
                    lhsT=kT_sb[DK * h:DK * (h + 1), lt * LTS:(lt + 1) * LTS],
                    rhs=qT_sb[DK * h:DK * (h + 1), qc * QC:(qc + 1) * QC],
                    start=True, stop=True)
            e = epool.tile([LTS, 2 * QC], mm_dt, tag="e", name="e")
            nc.scalar.activation(e, ss, EXP)
            return e

        def av_pair(v_sb, ps_att, e, lt, start, stop):
            for h in range(2):
                nc.tensor.matmul(ps_att[h],
                                 lhsT=v_sb[:, lt, h * VW:(h + 1) * VW],
                                 rhs=e[:, h * QC:(h + 1) * QC],
                                 start=start, stop=stop)

        def attention_chunk(n, qT_sb, kT_sb, v_sb, qc, work, slots=None):
            """Emit attention for one q-chunk; returns the accumulator PSUM
            pair. `slots` maps l-tile index -> thunks that MUST be emitted at
            that point (K/V production deadlines); `work` thunks are consumed
            evenly across the l-tile loop so the PE stream stays dense while
            ScalarE paces the exp pipeline."""
            slots = slots or {}
            ps_att = [atpool.tile([VW, QC], F32, tag=f"pa{h}",
                                  name=f"ps_att{h}") for h in range(2)]
            prev = None
            for lt in range(NLT):
                e = scores_pair(qT_sb, kT_sb, qc, lt)
                for t in slots.get(lt, ()):
                    t()
                if work:
                    take = -(-len(work) // (NLT - lt))
                    for _ in range(min(take, len(work))):
                        work.popleft()()
                if prev is not None:
                    av_pair(v_sb, ps_att, prev, lt - 1, start=(lt == 1),
                            stop=False)
                prev = e
            av_pair(v_sb, ps_att, prev, NLT - 1, start=(NLT == 1), stop=True)
            while work:
                work.popleft()()
            return ps_att

        def norm_thunks(ps_att):
            """Deferred softmax normalization for a finished accumulator
            pair. Returns (att, [A0, A1, C, D]): A* quick-release the PSUM
            banks, C builds the 1/den broadcast (fast reciprocal + K=1 PE
            matmul), D applies it."""
            att_raw = apool.tile([HP, QC], F32, tag="att_raw", name="att_raw")
            att = apool.tile([HP, QC], mm_dt, tag="attT", name="att")
            state = {}

            def tA(h):
                def f():
                    nc.vector.tensor_copy(att_raw[DK * h:DK * (h + 1), :],
                                          ps_att[h][0:DK, :])
                    den_f = apool.tile([1, QC], F32, tag=f"den{h}",
                                       name="den_f")
                    nc.vector.tensor_copy(den_f, ps_att[h][DK:VW, :])
                    state["den%d" % h] = den_f
                return f

            def tC():
                for h in range(2):
                    den_rf = apool.tile([1, QC], F32, tag=f"denr{h}",
                                        name="den_rf")
                    nc.vector.reciprocal_approx_fast(den_rf,
                                                     state["den%d" % h])
                    den_rr = apool.tile([1, QC], mm_dt, tag=f"denrr{h}",
                                        name="den_rr")
                    nc.vector.tensor_copy(den_rr, den_rf)
                    bcp = ppool.tile([DK, QC], F32, tag="pp", name="bc_ps")
                    nc.tensor.matmul(bcp, lhsT=ones_col_r, rhs=den_rr,
                                     start=True, stop=True)
                    state["bc%d" % h] = bcp

            def tD():
                for h in range(2):
                    nc.vector.tensor_mul(att[DK * h:DK * (h + 1), :],
                                         att_raw[DK * h:DK * (h + 1), :],
                                         state["bc%d" % h])

            return att, [tA(0), tA(1), tC, tD]

        def out_proj_thunks(n, att, qc):
            """out-projection chunk: 8 (MM + fp16-cast) thunks, DMA per
            q-tile."""
            box = {}
            thunks = []
            for j in range(QC // LTS):
                for half in range(2):
                    def t(j=j, half=half):
                        qt = qc * (QC // LTS) + j
                        if half == 0:
                            box[j] = opool.tile([LTS, D], out_dt, tag="osb",
                                                name="o_sb")
                        o_sb = box[j]
                        po = ppool.tile([LTS, QC], F32, tag="pp", name="po")
                        nc.tensor.matmul(
                            po, lhsT=att[:, j * LTS:(j + 1) * LTS],
                            rhs=wo_s[:, half * QC:(half + 1) * QC],
                            start=True, stop=True)
                        nc.vector.tensor_copy(
                            o_sb[:, half * QC:(half + 1) * QC], po)
                        if half == 1:
                            nc.scalar.dma_start(
                                out=O[n, qt * LTS:(qt + 1) * LTS, :],
                                in_=box.pop(j))
                    thunks.append(t)
            return thunks

        def body():
            seqs = []
            for n in range(NB):
                qT_sb = seq.tile([HP, T], mm_dt, tag="qT", name="qT_sb")
                kT_sb = seq.tile([HP, T], mm_dt, tag="kT", name="kT_sb")
                v_sb = seq.tile([LTS, NLT, 2 * VW], mm_dt, tag="v",
                                name="v_sb")
                nc.vector.tensor_copy(v_sb[:, :, DK:DK + 1], ones_lts)
                nc.vector.tensor_copy(v_sb[:, :, VW + DK:VW + DK + 1],
                                      ones_lts)
                seqs.append((qT_sb, kT_sb, v_sb))

            def kv_dma(n, c):
                return (dma_thunks(KT, n, c, ("k", n, c))
                        + dma_thunks(VT, n, c, ("v", n, c)))

            def kv_mm(n, c):
                # dense-block variant: PSUM->SBUF casts on idle ScalarE
                k = qk_mm_thunks(wk_s, seqs[n][1], c, ("k", n, c),
                                 eng="scalar")
                v = v_mm_thunks(seqs[n][2], c, ("v", n, c), eng="scalar")
                return k + v

            def kv_mm_slots(base, n, c):
                """Slot layout for one K/V chunk projection: k matmuls at
                `base`, casts + v at base+1/+2 (ready before l-tile 4c)."""
                k = qk_mm_thunks(wk_s, seqs[n][1], c, ("k", n, c))
                v = v_mm_thunks(seqs[n][2], c, ("v", n, c))
                return {base: k[:2], base + 1: [k[2]] + v[:3],
                        base + 2: v[3:]}

            def merge(*dicts):
                out = {}
                for d in dicts:
                    for k, v in d.items():
                        out.setdefault(k, []).extend(v)
                return out

            # --- startup: weights first (projections gate on them), then
            # chunk (0,0)'s inputs, then deep K/V prefetch of batch 0,
            # then a DENSE projection block: all of batch-0's K/V + the
            # first two q-chunks, back-to-back at full PE pstate. Dripping
            # over-budget projection work into the exp slack proved slower
            # (the l-loops have no real slack; drip = pstate thrash). ---
            load_w(wq_s, WQp)
            for t in dma_thunks(QT, 0, 0, ("q", 0, 0)):
                t()
            load_w(wk_s, WKp)
            for t in dma_thunks(KT, 0, 0, ("k", 0, 0)):
                t()
            load_w(wv_s, WVp)
            for t in dma_thunks(VT, 0, 0, ("v", 0, 0)):
                t()
            nc.sync.dma_start(out=wo_s, in_=WOp)
            for c in range(1, NQC):
                for t in kv_dma(0, c):
                    t()
            for t in dma_thunks(QT, 0, 1, ("q", 0, 1)):
                t()
            for t in qk_mm_thunks(wq_s, seqs[0][0], 0, ("q", 0, 0),
                                  eng="scalar"):
                t()
            for c in range(NQC):
                for t in kv_mm(0, c):
                    t()
            for t in qk_mm_thunks(wq_s, seqs[0][0], 1, ("q", 0, 1),
                                  eng="scalar"):
                t()

            # batch-1 K/V input DMAs prefetch across batch-0's chunks
            slots_for = {
                (0, 0): {3: kv_dma(1, 0)},
                (0, 1): {3: kv_dma(1, 1)},
                (0, 2): {3: kv_dma(1, 2)},
                (0, 3): {3: kv_dma(1, 3)},
            }

            pend_norm = None
            pend_out = None
            for n in range(NB):
                qT_sb, kT_sb, v_sb = seqs[n]
                if n == 1:
                    # dense inter-batch block: project all of batch-1's K/V
                    for c in range(NQC):
                        for t in kv_mm(1, c):
                            t()
                for qc in range(NQC):
                    # q-chunk projections roll two chunks ahead (chunks 0
                    # and 1 of each batch are projected before the batch)
                    la = (n, qc + 2) if qc + 2 < NQC else \
                         (n + 1, qc - 2) if n + 1 < NB else None
                    work = deque()
                    if pend_norm:
                        work.extend(pend_norm[:2])      # PSUM quick-release
                    if la:
                        work.extend(dma_thunks(QT, la[0], la[1],
                                               ("q",) + la))
                    if pend_norm:
                        work.extend(pend_norm[2:])      # bc + mul
                    if pend_out is not None:
                        work.extend(out_proj_thunks(pend_out[2], pend_out[0],
                                                    pend_out[1]))
                    if la:
                        work.extend(qk_mm_thunks(
                            wq_s, seqs[la[0]][0], la[1], ("q",) + la))
                    ps_att = attention_chunk(n, qT_sb, kT_sb, v_sb, qc, work,
                                             slots_for.get((n, qc)))
                    att, pend_norm = norm_thunks(ps_att)
                    pend_out = (att, qc, n)
            for t in pend_norm:
                t()
            for t in out_proj_thunks(pend_out[2], pend_out[0], pend_out[1]):
                t()

        body()

    nc.compile()
    return nc


_CACHED = {}


def _get_program(key=("bf16",)):
    if key not in _CACHED:
        _CACHED[key] = build_program()
    return _CACHED[key]


def prep_inputs(Q, K, V, WQ, WK, WV, WO, bO):
    """Host-side shard prep: transposes + per-core weight slices."""
    import ml_dtypes
    wire = ml_dtypes.bfloat16
    Q = np.asarray(Q, dtype=np.float32)
    K = np.asarray(K, dtype=np.float32)
    V = np.asarray(V, dtype=np.float32)
    WQ = np.asarray(WQ, dtype=np.float32)
    WK = np.asarray(WK, dtype=np.float32)
    WV = np.asarray(WV, dtype=np.float32)
    WO = np.asarray(WO, dtype=np.float32)
    def blockT(X):
        # [N, T, D] -> transpose -> [N, D, T] -> blocks [N, NCK, NQC, CK, QC]
        Xt = np.swapaxes(X, 1, 2).reshape(NB, NCK, CK, NQC, QC)
        return np.ascontiguousarray(
            Xt.transpose(0, 1, 3, 2, 4)).astype(wire)

    QT = blockT(Q)
    KT = blockT(K)
    VT = blockT(V)
    scale = 1.0 / math.sqrt(DK)
    in_maps = []
    for p in range(N_CORES):
        sl = slice(HP * p, HP * (p + 1))
        in_maps.append({
            "QT": QT, "KT": KT, "VT": VT,
            "WQp": (np.ascontiguousarray(WQ[:, sl]) * scale).astype(wire),
            "WKp": np.ascontiguousarray(WK[:, sl]).astype(wire),
            "WVp": np.ascontiguousarray(WV[:, sl]).astype(wire),
            "WOp": np.ascontiguousarray(WO[sl, :]).astype(wire),
        })
    return in_maps


def kernel(Q, K, V, WQ, WK, WV, WO, bO):
    nc = _get_program()
    in_maps = prep_inputs(Q, K, V, WQ, WK, WV, WO, bO)
    res = run_bass_kernel_spmd(nc, in_maps, list(range(N_CORES)))
    acc = np.zeros((NB, T, D), np.float32)
    for p in range(N_CORES):
        acc += res.results[p]["O"].astype(np.float32)
    return acc + np.asarray(bO, dtype=np.float32)
